# revision 17
# baseline (speedup 1.0000x reference)
import sys, os
sys.path.insert(0, "/opt/trn_rl_repo")
import numpy as np
import ml_dtypes
from contextlib import ExitStack

import concourse.bass as bass
import concourse.tile as tile
from concourse import bacc, mybir
from concourse.bass_utils import run_bass_kernel_spmd

BF16 = ml_dtypes.bfloat16
F32 = np.float32

B, C, T = 4, 512, 16384
DILATIONS = (1, 2, 4)
SLOPE = 0.1
NCORES = 8
HALF = T // 2            # 8192 per core
H = 128                  # halo per side
SL = HALF + 2 * H        # 8448 slice length
TC = 2048                # chunk kept size
E = TC + 2 * H           # 2304 chunk buffer extent
NCHUNK = HALF // TC      # 4
NT = E // 128            # 18 t-tiles per chunk buffer
K3WIN = [(1, 512), (513, 512), (1025, 512), (1537, 512), (2049, 254)]

_NC = None


def _build(with_bias=True):
    nc = bacc.Bacc("TRN2", target_bir_lowering=False, debug=False, num_devices=NCORES)
    dt = mybir.dt

    xs = nc.dram_tensor("xs", [C, SL], dt.bfloat16, kind="ExternalInput").ap()
    mks = nc.dram_tensor("mks", [12 * 128, SL], dt.bfloat16, kind="ExternalInput").ap()
    em = nc.dram_tensor("em", [1, 2 * NCHUNK], dt.float32, kind="ExternalInput").ap()
    w1 = nc.dram_tensor("w1", [128, 36, 512], dt.bfloat16, kind="ExternalInput").ap()
    wa = nc.dram_tensor("wa", [128, 36, 512], dt.bfloat16, kind="ExternalInput").ap()
    bsum = nc.dram_tensor("bsum", [1, 3 * 512], dt.bfloat16, kind="ExternalInput").ap()
    ba_r = nc.dram_tensor("ba_r", [1, 3 * 512], dt.bfloat16, kind="ExternalInput").ap()
    eye = nc.dram_tensor("eye", [128, 128], dt.bfloat16, kind="ExternalInput").ap()
    out = nc.dram_tensor("out", [C, HALF], dt.bfloat16, kind="ExternalOutput").ap()

    with tile.TileContext(nc) as tc:
        with ExitStack() as ctx:
            consts = ctx.enter_context(tc.tile_pool(name="consts", bufs=1))
            xpool = ctx.enter_context(tc.tile_pool(name="xpool", bufs=3))
            xtpool = ctx.enter_context(tc.tile_pool(name="xtpool", bufs=1))
            mpool = ctx.enter_context(tc.tile_pool(name="mpool", bufs=4))
            wapool = ctx.enter_context(tc.tile_pool(name="wapool", bufs=2))
            ypool = ctx.enter_context(tc.tile_pool(name="ypool", bufs=8))
            ztpool = ctx.enter_context(tc.tile_pool(name="ztpool", bufs=3))
            zpool = ctx.enter_context(tc.tile_pool(name="zpool", bufs=2))
            psA = ctx.enter_context(tc.tile_pool(name="psA", bufs=3, space="PSUM"))
            psB = ctx.enter_context(tc.tile_pool(name="psB", bufs=5, space="PSUM"))

            # ---- constants ----
            w1_sb = consts.tile([128, 36, 512], dt.bfloat16)
            nc.sync.dma_start(out=w1_sb[:], in_=w1)
            eye_sb = consts.tile([128, 128], dt.bfloat16)
            nc.sync.dma_start(out=eye_sb[:], in_=eye)
            bsum_sb = consts.tile([1, 3 * 512], dt.bfloat16)
            nc.sync.dma_start(out=bsum_sb[:], in_=bsum)
            ba_sb = consts.tile([1, 3 * 512], dt.bfloat16)
            nc.sync.dma_start(out=ba_sb[:], in_=ba_r)
            em_sb = consts.tile([128, 2 * NCHUNK], dt.float32)
            nc.gpsimd.dma_start(
                out=em_sb[:],
                in_=bass.AP(tensor=em.tensor, offset=0, ap=[[0, 128], [1, 2 * NCHUNK]]),
            )
            ones128 = consts.tile([1, 128], dt.bfloat16)
            nc.vector.memset(ones128[:], 1.0)
            ones512 = consts.tile([1, 512], dt.bfloat16)
            nc.vector.memset(ones512[:], 1.0)

            for ck in range(NCHUNK):
                cb = ck * TC

                x_cur = xpool.tile([128, 4, E], dt.bfloat16, tag="x")
                nc.sync.dma_start(
                    out=x_cur[:],
                    in_=xs[:, cb : cb + E].rearrange("(a p) t -> p a t", p=128),
                )

                xt = xtpool.tile([128, 4, E], dt.bfloat16, tag="xt")
                nc.scalar.activation(
                    out=xt[:], in_=x_cur[:],
                    func=mybir.ActivationFunctionType.Prelu, alpha=SLOPE,
                )

                for l in range(3):
                    masks = {}
                    for i, nm in enumerate(("P_diag", "P_sub", "F_diag", "F_sup")):
                        mk = mpool.tile([128, E], dt.bfloat16, tag="mask")
                        r0 = (l * 4 + i) * 128
                        nc.sync.dma_start(out=mk[:], in_=mks[r0 : r0 + 128, cb : cb + E])
                        masks[nm] = mk
                    wa_sb = wapool.tile([128, 12, 512], dt.bfloat16, tag="wa")
                    nc.sync.dma_start(out=wa_sb[:], in_=wa[:, l * 12 : (l + 1) * 12, :])

                    ycT = [None] * NT
                    ypS = [None] * NT
                    yfS = [None] * NT
                    zTs = [None] * NT

                    def conv_step(j):
                        yc = psA.tile([128, 512], dt.float32, tag="ycT")
                        yp = psB.tile([128, 512], dt.float32, tag="ps")
                        yf = psB.tile([128, 512], dt.float32, tag="ps")
                        for a in range(4):
                            lhs = xt[:, a, 128 * j : 128 * (j + 1)]
                            st = a == 0
                            nc.tensor.matmul(yc[:], lhs, w1_sb[:, (l * 3 + 0) * 4 + a, :], start=st, stop=False)
                            nc.tensor.matmul(yp[:], lhs, w1_sb[:, (l * 3 + 1) * 4 + a, :], start=st, stop=a == 3)
                            nc.tensor.matmul(yf[:], lhs, w1_sb[:, (l * 3 + 2) * 4 + a, :], start=st, stop=a == 3)
                        if with_bias:
                            nc.tensor.matmul(yc[:], ones128[:], bsum_sb[:, l * 512 : (l + 1) * 512], start=False, stop=False)
                        ycT[j] = yc
                        yp_s = ypool.tile([128, 512], dt.bfloat16, tag="yps")
                        nc.vector.tensor_copy(out=yp_s[:], in_=yp[:])
                        yf_s = ypool.tile([128, 512], dt.bfloat16, tag="yps")
                        nc.scalar.activation(out=yf_s[:], in_=yf[:], func=mybir.ActivationFunctionType.Copy)
                        ypS[j] = yp_s
                        yfS[j] = yf_s

                    def gather_step(j):
                        tj = slice(128 * j, 128 * (j + 1))
                        if j > 0:
                            nc.tensor.matmul(ycT[j][:], masks["P_sub"][:, tj], ypS[j - 1][:], start=False, stop=False)
                        nc.tensor.matmul(ycT[j][:], masks["P_diag"][:, tj], ypS[j][:], start=False, stop=False)
                        nc.tensor.matmul(ycT[j][:], masks["F_diag"][:, tj], yfS[j][:], start=False, stop=j == NT - 1)
                        if j < NT - 1:
                            nc.tensor.matmul(ycT[j][:], masks["F_sup"][:, tj], yfS[j + 1][:], start=False, stop=False)
                        zt = ztpool.tile([128, 512], dt.bfloat16, tag="zt")
                        nc.scalar.activation(out=zt[:], in_=ycT[j][:], func=mybir.ActivationFunctionType.Prelu, alpha=SLOPE)
                        zTs[j] = zt

                    z = zpool.tile([128, 4, E], dt.bfloat16, tag="z")

                    def transpose_step(j):
                        nc.scalar.dma_start_transpose(out=z[:, :, 128 * j : 128 * (j + 1)], in_=zTs[j][:])
                        if j == 0:
                            for m in range(4):
                                nc.vector.tensor_scalar(
                                    out=z[:, m, H - 1 : H], in0=z[:, m, H - 1 : H],
                                    scalar1=em_sb[:, 2 * ck : 2 * ck + 1], scalar2=None,
                                    op0=mybir.AluOpType.mult,
                                )
                        if j == NT - 1:
                            for m in range(4):
                                nc.vector.tensor_scalar(
                                    out=z[:, m, H + TC : H + TC + 1], in0=z[:, m, H + TC : H + TC + 1],
                                    scalar1=em_sb[:, 2 * ck + 1 : 2 * ck + 2], scalar2=None,
                                    op0=mybir.AluOpType.mult,
                                )

                    for j in range(NT):
                        conv_step(j)
                        if j >= 1:
                            gather_step(j - 1)
                            transpose_step(j - 1)
                    gather_step(NT - 1)
                    transpose_step(NT - 1)

                    # ---- k=3 conv + residual (+ next-layer lrelu per window) ----
                    x_next = xpool.tile([128, 4, E], dt.bfloat16, tag="x")
                    last = l == 2
                    if not last:
                        xt_n = xtpool.tile([128, 4, E], dt.bfloat16, tag="xt")
                        for m in range(4):
                            nc.vector.memset(xt_n[:, m, 0:1], 0.0)
                            nc.vector.memset(xt_n[:, m, E - 1 : E], 0.0)
                    for grp in (K3WIN,):
                        for m in range(4):
                            pks = []
                            for _wi in range(len(grp)):
                                pk_w = psB.tile([128, 512], dt.float32, tag="ps")
                                pks.append(pk_w)
                            for ki, (tau, a) in enumerate([(t_, a_) for t_ in range(3) for a_ in range(4)]):
                                lhs = wa_sb[:, tau * 4 + a, 128 * m : 128 * (m + 1)]
                                for wi, (w0, wn) in enumerate(grp):
                                    rhs = z[:, a, w0 + tau - 1 : w0 + tau - 1 + wn]
                                    nc.tensor.matmul(pks[wi][:, 0:wn], lhs, rhs, start=ki == 0,
                                                     stop=(ki == 11) and not with_bias)
                            for wi, (w0, wn) in enumerate(grp):
                                pk = pks[wi]
                                if with_bias:
                                    nc.tensor.matmul(
                                        pk[:, 0:wn], ba_sb[:, l * 512 + 128 * m : l * 512 + 128 * (m + 1)],
                                        ones512[:, 0:wn], start=False, stop=True,
                                    )
                                nc.vector.tensor_tensor(
                                    out=x_next[:, m, w0 : w0 + wn], in0=pk[:, 0:wn],
                                    in1=x_cur[:, m, w0 : w0 + wn], op=mybir.AluOpType.add,
                                )
                                if not last:
                                    nc.scalar.activation(
                                        out=xt_n[:, m, w0 : w0 + wn], in_=x_next[:, m, w0 : w0 + wn],
                                        func=mybir.ActivationFunctionType.Prelu, alpha=SLOPE,
                                    )
                    x_cur = x_next
                    if not last:
                        xt = xt_n

                nc.sync.dma_start(
                    out=out[:, ck * TC : (ck + 1) * TC].rearrange("(a p) t -> p a t", p=128),
                    in_=x_cur[:, :, H : H + TC],
                )

    nc.compile()
    return nc


def _host_inputs(x, d, Wc, bc, Wp, bp, Wf, bf, Wa, ba):
    x = np.asarray(x, dtype=F32)
    d = np.asarray(d, dtype=F32)
    Wc, Wp, Wf = (np.asarray(w, dtype=F32) for w in (Wc, Wp, Wf))
    Wa = np.asarray(Wa, dtype=F32)
    bc, bp, bf, ba = (np.asarray(v, dtype=F32) for v in (bc, bp, bf, ba))

    w1 = np.empty((128, 36, 512), dtype=BF16)
    wa = np.empty((128, 36, 512), dtype=BF16)
    for l in range(3):
        for cv, W in enumerate((Wc, Wp, Wf)):
            wt = W[l].T.astype(BF16)
            for a in range(4):
                w1[:, (l * 3 + cv) * 4 + a, :] = wt[a * 128 : (a + 1) * 128, :]
        for tau in range(3):
            wt = Wa[l][:, :, tau].T.astype(BF16)
            for a in range(4):
                wa[:, (l * 3 + tau) * 4 + a, :] = wt[a * 128 : (a + 1) * 128, :]
    bsum = (bc + bp + bf).reshape(1, -1).astype(BF16)
    ba_r = ba.reshape(1, -1).astype(BF16)
    eye = np.eye(128, dtype=BF16)

    p_ar = np.arange(128, dtype=np.int64)[:, None]
    tilebase = 128 * (np.arange(SL, dtype=np.int64) // 128)[None, :]

    in_maps = []
    for core in range(NCORES):
        b, h = core // 2, core % 2
        g0 = h * HALF
        lo = g0 - H
        xsl = np.zeros((C, SL), dtype=BF16)
        dsl = np.zeros((1, SL), dtype=F32)
        s0 = max(0, lo)
        s1 = min(T, g0 + HALF + H)
        xsl[:, s0 - lo : s1 - lo] = x[b, :, s0:s1].astype(BF16)
        dsl[:, s0 - lo : s1 - lo] = d[b, :, s0:s1]

        tg = (np.arange(SL, dtype=np.float64) + lo).astype(F32)
        cl = float(max(0, lo))
        chq = float(min(T - 1, g0 + HALF + H - 1))
        mks = np.zeros((12 * 128, SL), dtype=BF16)
        for l in range(3):
            dil = np.float32(DILATIONS[l])
            dila = (dsl[0] * dil).astype(F32)
            for gi, sgn in ((0, np.float32(-1.0)), (1, np.float32(1.0))):
                u = (tg + sgn * dila).astype(F32)
                idxg = np.clip(np.round(u), cl, chq).astype(np.int64)
                rel = (idxg - lo)[None, :] - tilebase
                diag = (rel == p_ar).astype(BF16)
                off = (rel == (p_ar - 128)).astype(BF16) if gi == 0 else (rel == (p_ar + 128)).astype(BF16)
                i_diag = l * 4 + (0 if gi == 0 else 2)
                i_off = l * 4 + (1 if gi == 0 else 3)
                mks[i_diag * 128 : (i_diag + 1) * 128, :] = diag
                mks[i_off * 128 : (i_off + 1) * 128, :] = off

        em = np.ones((1, 2 * NCHUNK), dtype=F32)
        if h == 0:
            em[0, 0] = 0.0
        if h == 1:
            em[0, 2 * NCHUNK - 1] = 0.0
        in_maps.append(
            dict(xs=xsl, mks=mks, em=em, w1=w1, wa=wa, bsum=bsum, ba_r=ba_r, eye=eye)
        )
    return in_maps


_NC_BIAS = None


def kernel(**inputs):
    global _NC, _NC_BIAS
    wb = any(np.any(np.asarray(inputs[k])) for k in ("bc", "bp", "bf", "ba"))
    if _NC is None or _NC_BIAS != wb:
        _NC = _build(with_bias=wb)
        _NC_BIAS = wb
    in_maps = _host_inputs(**inputs)
    res = run_bass_kernel_spmd(_NC, in_maps, core_ids=list(range(NCORES)), trace=False)
    out = np.empty((B, C, T), dtype=F32)
    for core in range(NCORES):
        b, h = core // 2, core % 2
        out[b, :, h * HALF : (h + 1) * HALF] = np.asarray(res.results[core]["out"]).astype(F32)
    return out


# revision 18
# speedup vs baseline: 1.2639x; 1.2639x over previous
import sys, os
sys.path.insert(0, "/opt/trn_rl_repo")
import numpy as np
import ml_dtypes
from contextlib import ExitStack

import concourse.bass as bass
import concourse.tile as tile
from concourse import bacc, mybir
from concourse.bass_utils import run_bass_kernel_spmd

BF16 = ml_dtypes.bfloat16
F32 = np.float32

B, C, T = 4, 512, 16384
DILATIONS = (1, 2, 4)
SLOPE = 0.1
NCORES = 8
HALF = T // 2            # 8192 per core
H = 128                  # halo per side
SL = HALF + 2 * H        # 8448 slice length
TC = 2048                # chunk kept size
E = TC + 2 * H           # 2304 chunk buffer extent
NCHUNK = HALF // TC      # 4
NT = E // 128            # 18 t-tiles per chunk buffer
K3WIN = [(1, 512), (513, 512), (1025, 512), (1537, 512), (2049, 254)]

_NC = None


def _build(with_bias=True):
    nc = bacc.Bacc("TRN2", target_bir_lowering=False, debug=False, num_devices=NCORES)
    dt = mybir.dt

    xs = nc.dram_tensor("xs", [C, SL], dt.bfloat16, kind="ExternalInput").ap()
    mks = nc.dram_tensor("mks", [12 * 128, SL], dt.bfloat16, kind="ExternalInput").ap()
    em = nc.dram_tensor("em", [1, 2 * NCHUNK], dt.float32, kind="ExternalInput").ap()
    w1 = nc.dram_tensor("w1", [128, 36, 512], dt.bfloat16, kind="ExternalInput").ap()
    wa = nc.dram_tensor("wa", [128, 36, 512], dt.bfloat16, kind="ExternalInput").ap()
    bsum = nc.dram_tensor("bsum", [1, 3 * 512], dt.bfloat16, kind="ExternalInput").ap()
    ba_r = nc.dram_tensor("ba_r", [1, 3 * 512], dt.bfloat16, kind="ExternalInput").ap()
    eye = nc.dram_tensor("eye", [128, 128], dt.bfloat16, kind="ExternalInput").ap()
    out = nc.dram_tensor("out", [C, HALF], dt.bfloat16, kind="ExternalOutput").ap()

    with tile.TileContext(nc) as tc:
        with ExitStack() as ctx:
            consts = ctx.enter_context(tc.tile_pool(name="consts", bufs=1))
            xpool = ctx.enter_context(tc.tile_pool(name="xpool", bufs=3))
            xtpool = ctx.enter_context(tc.tile_pool(name="xtpool", bufs=1))
            mpool = ctx.enter_context(tc.tile_pool(name="mpool", bufs=4))
            wapool = ctx.enter_context(tc.tile_pool(name="wapool", bufs=2))
            ypool = ctx.enter_context(tc.tile_pool(name="ypool", bufs=8))
            ztpool = ctx.enter_context(tc.tile_pool(name="ztpool", bufs=3))
            zpool = ctx.enter_context(tc.tile_pool(name="zpool", bufs=2))
            psA = ctx.enter_context(tc.tile_pool(name="psA", bufs=3, space="PSUM"))
            psB = ctx.enter_context(tc.tile_pool(name="psB", bufs=5, space="PSUM"))

            # ---- constants ----
            w1_sb = consts.tile([128, 36, 512], dt.bfloat16)
            nc.sync.dma_start(out=w1_sb[:], in_=w1)
            eye_sb = consts.tile([128, 128], dt.bfloat16)
            nc.sync.dma_start(out=eye_sb[:], in_=eye)
            bsum_sb = consts.tile([1, 3 * 512], dt.bfloat16)
            nc.sync.dma_start(out=bsum_sb[:], in_=bsum)
            ba_sb = consts.tile([1, 3 * 512], dt.bfloat16)
            nc.sync.dma_start(out=ba_sb[:], in_=ba_r)
            em_sb = consts.tile([128, 2 * NCHUNK], dt.float32)
            nc.gpsimd.dma_start(
                out=em_sb[:],
                in_=bass.AP(tensor=em.tensor, offset=0, ap=[[0, 128], [1, 2 * NCHUNK]]),
            )
            ones128 = consts.tile([1, 128], dt.bfloat16)
            nc.vector.memset(ones128[:], 1.0)
            ones512 = consts.tile([1, 512], dt.bfloat16)
            nc.vector.memset(ones512[:], 1.0)

            for ck in range(NCHUNK):
                cb = ck * TC

                x_cur = xpool.tile([128, 4, E], dt.bfloat16, tag="x")
                nc.sync.dma_start(
                    out=x_cur[:],
                    in_=xs[:, cb : cb + E].rearrange("(a p) t -> p a t", p=128),
                )

                xt = xtpool.tile([128, 4, E], dt.bfloat16, tag="xt")
                nc.scalar.activation(
                    out=xt[:], in_=x_cur[:],
                    func=mybir.ActivationFunctionType.Prelu, alpha=SLOPE,
                )

                for l in range(3):
                    masks = {}
                    for i, nm in enumerate(("P_diag", "P_sub", "F_diag", "F_sup")):
                        mk = mpool.tile([128, E], dt.bfloat16, tag="mask")
                        r0 = (l * 4 + i) * 128
                        nc.sync.dma_start(out=mk[:], in_=mks[r0 : r0 + 128, cb : cb + E])
                        masks[nm] = mk
                    wa_sb = wapool.tile([128, 12, 512], dt.bfloat16, tag="wa")
                    nc.sync.dma_start(out=wa_sb[:], in_=wa[:, l * 12 : (l + 1) * 12, :])

                    ycT = [None] * NT
                    ypS = [None] * NT
                    yfS = [None] * NT
                    zTs = [None] * NT

                    def conv_step(j):
                        yc = psA.tile([128, 512], dt.float32, tag="ycT")
                        yp = psB.tile([128, 512], dt.float32, tag="ps")
                        yf = psB.tile([128, 512], dt.float32, tag="ps")
                        for a in range(4):
                            lhs = xt[:, a, 128 * j : 128 * (j + 1)]
                            st = a == 0
                            nc.tensor.matmul(yc[:], lhs, w1_sb[:, (l * 3 + 0) * 4 + a, :], start=st, stop=False)
                            nc.tensor.matmul(yp[:], lhs, w1_sb[:, (l * 3 + 1) * 4 + a, :], start=st, stop=a == 3)
                            nc.tensor.matmul(yf[:], lhs, w1_sb[:, (l * 3 + 2) * 4 + a, :], start=st, stop=a == 3)
                        if with_bias:
                            nc.tensor.matmul(yc[:], ones128[:], bsum_sb[:, l * 512 : (l + 1) * 512], start=False, stop=False)
                        ycT[j] = yc
                        yp_s = ypool.tile([128, 512], dt.bfloat16, tag="yps")
                        nc.vector.tensor_copy(out=yp_s[:], in_=yp[:])
                        yf_s = ypool.tile([128, 512], dt.bfloat16, tag="yps")
                        nc.scalar.activation(out=yf_s[:], in_=yf[:], func=mybir.ActivationFunctionType.Copy)
                        ypS[j] = yp_s
                        yfS[j] = yf_s

                    def gather_step(j):
                        tj = slice(128 * j, 128 * (j + 1))
                        if j > 0:
                            nc.tensor.matmul(ycT[j][:], masks["P_sub"][:, tj], ypS[j - 1][:], start=False, stop=False)
                        nc.tensor.matmul(ycT[j][:], masks["P_diag"][:, tj], ypS[j][:], start=False, stop=False)
                        nc.tensor.matmul(ycT[j][:], masks["F_diag"][:, tj], yfS[j][:], start=False, stop=j == NT - 1)
                        if j < NT - 1:
                            nc.tensor.matmul(ycT[j][:], masks["F_sup"][:, tj], yfS[j + 1][:], start=False, stop=False)
                        zt = ztpool.tile([128, 512], dt.bfloat16, tag="zt")
                        nc.scalar.activation(out=zt[:], in_=ycT[j][:], func=mybir.ActivationFunctionType.Prelu, alpha=SLOPE)
                        zTs[j] = zt

                    z = zpool.tile([128, 4, E], dt.bfloat16, tag="z")

                    def transpose_step(j):
                        nc.sync.dma_start_transpose(out=z[:, :, 128 * j : 128 * (j + 1)], in_=zTs[j][:])
                        if j == 0:
                            for m in range(4):
                                nc.vector.tensor_scalar(
                                    out=z[:, m, H - 1 : H], in0=z[:, m, H - 1 : H],
                                    scalar1=em_sb[:, 2 * ck : 2 * ck + 1], scalar2=None,
                                    op0=mybir.AluOpType.mult,
                                )
                        if j == NT - 1:
                            for m in range(4):
                                nc.vector.tensor_scalar(
                                    out=z[:, m, H + TC : H + TC + 1], in0=z[:, m, H + TC : H + TC + 1],
                                    scalar1=em_sb[:, 2 * ck + 1 : 2 * ck + 2], scalar2=None,
                                    op0=mybir.AluOpType.mult,
                                )

                    for j in range(NT):
                        conv_step(j)
                        if j >= 1:
                            gather_step(j - 1)
                            transpose_step(j - 1)
                    gather_step(NT - 1)
                    transpose_step(NT - 1)

                    # ---- k=3 conv + residual (+ next-layer lrelu per window) ----
                    x_next = xpool.tile([128, 4, E], dt.bfloat16, tag="x")
                    last = l == 2
                    if not last:
                        xt_n = xtpool.tile([128, 4, E], dt.bfloat16, tag="xt")
                        for m in range(4):
                            nc.vector.memset(xt_n[:, m, 0:1], 0.0)
                            nc.vector.memset(xt_n[:, m, E - 1 : E], 0.0)
                    for grp in (K3WIN,):
                        for m in range(4):
                            pks = []
                            for _wi in range(len(grp)):
                                pk_w = psB.tile([128, 512], dt.float32, tag="ps")
                                pks.append(pk_w)
                            for ki, (tau, a) in enumerate([(t_, a_) for t_ in range(3) for a_ in range(4)]):
                                lhs = wa_sb[:, tau * 4 + a, 128 * m : 128 * (m + 1)]
                                for wi, (w0, wn) in enumerate(grp):
                                    rhs = z[:, a, w0 + tau - 1 : w0 + tau - 1 + wn]
                                    nc.tensor.matmul(pks[wi][:, 0:wn], lhs, rhs, start=ki == 0,
                                                     stop=(ki == 11) and not with_bias)
                            for wi, (w0, wn) in enumerate(grp):
                                pk = pks[wi]
                                if with_bias:
                                    nc.tensor.matmul(
                                        pk[:, 0:wn], ba_sb[:, l * 512 + 128 * m : l * 512 + 128 * (m + 1)],
                                        ones512[:, 0:wn], start=False, stop=True,
                                    )
                                nc.vector.tensor_tensor(
                                    out=x_next[:, m, w0 : w0 + wn], in0=pk[:, 0:wn],
                                    in1=x_cur[:, m, w0 : w0 + wn], op=mybir.AluOpType.add,
                                )
                                if not last:
                                    nc.scalar.activation(
                                        out=xt_n[:, m, w0 : w0 + wn], in_=x_next[:, m, w0 : w0 + wn],
                                        func=mybir.ActivationFunctionType.Prelu, alpha=SLOPE,
                                    )
                    x_cur = x_next
                    if not last:
                        xt = xt_n

                nc.sync.dma_start(
                    out=out[:, ck * TC : (ck + 1) * TC].rearrange("(a p) t -> p a t", p=128),
                    in_=x_cur[:, :, H : H + TC],
                )

    nc.compile()
    return nc


def _host_inputs(x, d, Wc, bc, Wp, bp, Wf, bf, Wa, ba):
    x = np.asarray(x, dtype=F32)
    d = np.asarray(d, dtype=F32)
    Wc, Wp, Wf = (np.asarray(w, dtype=F32) for w in (Wc, Wp, Wf))
    Wa = np.asarray(Wa, dtype=F32)
    bc, bp, bf, ba = (np.asarray(v, dtype=F32) for v in (bc, bp, bf, ba))

    w1 = np.empty((128, 36, 512), dtype=BF16)
    wa = np.empty((128, 36, 512), dtype=BF16)
    for l in range(3):
        for cv, W in enumerate((Wc, Wp, Wf)):
            wt = W[l].T.astype(BF16)
            for a in range(4):
                w1[:, (l * 3 + cv) * 4 + a, :] = wt[a * 128 : (a + 1) * 128, :]
        for tau in range(3):
            wt = Wa[l][:, :, tau].T.astype(BF16)
            for a in range(4):
                wa[:, (l * 3 + tau) * 4 + a, :] = wt[a * 128 : (a + 1) * 128, :]
    bsum = (bc + bp + bf).reshape(1, -1).astype(BF16)
    ba_r = ba.reshape(1, -1).astype(BF16)
    eye = np.eye(128, dtype=BF16)

    p_ar = np.arange(128, dtype=np.int64)[:, None]
    tilebase = 128 * (np.arange(SL, dtype=np.int64) // 128)[None, :]

    in_maps = []
    for core in range(NCORES):
        b, h = core // 2, core % 2
        g0 = h * HALF
        lo = g0 - H
        xsl = np.zeros((C, SL), dtype=BF16)
        dsl = np.zeros((1, SL), dtype=F32)
        s0 = max(0, lo)
        s1 = min(T, g0 + HALF + H)
        xsl[:, s0 - lo : s1 - lo] = x[b, :, s0:s1].astype(BF16)
        dsl[:, s0 - lo : s1 - lo] = d[b, :, s0:s1]

        tg = (np.arange(SL, dtype=np.float64) + lo).astype(F32)
        cl = float(max(0, lo))
        chq = float(min(T - 1, g0 + HALF + H - 1))
        mks = np.zeros((12 * 128, SL), dtype=BF16)
        for l in range(3):
            dil = np.float32(DILATIONS[l])
            dila = (dsl[0] * dil).astype(F32)
            for gi, sgn in ((0, np.float32(-1.0)), (1, np.float32(1.0))):
                u = (tg + sgn * dila).astype(F32)
                idxg = np.clip(np.round(u), cl, chq).astype(np.int64)
                rel = (idxg - lo)[None, :] - tilebase
                diag = (rel == p_ar).astype(BF16)
                off = (rel == (p_ar - 128)).astype(BF16) if gi == 0 else (rel == (p_ar + 128)).astype(BF16)
                i_diag = l * 4 + (0 if gi == 0 else 2)
                i_off = l * 4 + (1 if gi == 0 else 3)
                mks[i_diag * 128 : (i_diag + 1) * 128, :] = diag
                mks[i_off * 128 : (i_off + 1) * 128, :] = off

        em = np.ones((1, 2 * NCHUNK), dtype=F32)
        if h == 0:
            em[0, 0] = 0.0
        if h == 1:
            em[0, 2 * NCHUNK - 1] = 0.0
        in_maps.append(
            dict(xs=xsl, mks=mks, em=em, w1=w1, wa=wa, bsum=bsum, ba_r=ba_r, eye=eye)
        )
    return in_maps


_NC_BIAS = None


def kernel(**inputs):
    global _NC, _NC_BIAS
    wb = any(np.any(np.asarray(inputs[k])) for k in ("bc", "bp", "bf", "ba"))
    if _NC is None or _NC_BIAS != wb:
        _NC = _build(with_bias=wb)
        _NC_BIAS = wb
    in_maps = _host_inputs(**inputs)
    res = run_bass_kernel_spmd(_NC, in_maps, core_ids=list(range(NCORES)), trace=False)
    out = np.empty((B, C, T), dtype=F32)
    for core in range(NCORES):
        b, h = core // 2, core % 2
        out[b, :, h * HALF : (h + 1) * HALF] = np.asarray(res.results[core]["out"]).astype(F32)
    return out
